# revision 12
# baseline (speedup 1.0000x reference)
"""v26 Trainium2 Bass kernel for an attention-style graph convolution (GAT).

Full staircase split. With i sorted by s1 (stratified mod-4 over slabs) and
j sorted by s2 (stratified mod-2 over halves), each (chunk c, i-block it)
tile is one of three exact types:
  Q (u == 1,        bmax_es1b * es2m_c <= 1):  acc[it] += m8.T @ gQ_c
  P (u == es1b*es2m, bmin_es1b * es2m_c >= 1): acc[it] += m8.T @ gP_c,
     with the per-row factor es1b_i applied ONCE by an in-place DVE scale
     of the PSUM accumulator at the block's P->non-P transition
  band (otherwise): u = max(es1b*es2m_c, 1) (ts); n = u*mt (tt);
                    acc[it] += n.T @ g2_c
where mt = m*es2a_j (fp16, streamed only for the band), m8 = binary fp8,
gQ = es2a*g2, gP = es2f*g2. Chunks are processed in DESCENDING c (s2) so
each block's P-chunks come first (suffix in c = prefix in processing),
making the single in-place scale exact. Tables kq/kp are conservative
(min/max over the 8 cores) so one SPMD program serves all cores.
Host sums j-half core pairs, un-permutes rows, divides, applies elu.
"""

import ml_dtypes
import numpy as np

import concourse.bacc as bacc
import concourse.bass as bass
import concourse.mybir as mybir
import concourse.tile as tile
from concourse import bass_utils

F32 = mybir.dt.float32
BF16 = mybir.dt.bfloat16
FP16 = mybir.dt.float16
FP8 = mybir.dt.float8e4
OP = mybir.AluOpType

N = 8192
K = 256
F = 128
ALPHA = 0.2
NCORES = 8
MI = 2048
MJ = 4096
P = 128
NJ = MJ // P      # 32 j-chunks
NIT = MI // P     # 16 i-blocks
LAG = 3
GW = F + 1


def _broadcast_ap(row_ap, nparts):
    return bass.AP(
        tensor=row_ap.tensor,
        offset=row_ap.offset,
        ap=[[0, nparts]] + [list(d) for d in row_ap.ap],
    )


def band_offsets(kq, kp):
    offs, tot = [], 0
    for c in range(NJ):
        offs.append(tot)
        tot += (kp[c] - kq[c]) * P
    return offs, max(tot, P)


def m8_groups():
    # descending processing order; tapered group sizes at both ends
    gs, c = [], NJ - 1
    sizes = [1] + [4] * 7 + [1, 1, 1]
    for s in sizes:
        gs.append(list(range(c, c - s, -1)))
        c -= s
    assert c == -1
    return gs


def build_program(kq, kp):
    BW = max(1, max(p - q for p, q in zip(kp, kq)))
    offs, BSUM = band_offsets(kq, kp)
    groups = m8_groups()
    nc = bacc.Bacc("TRN2", target_bir_lowering=False)

    band_d = nc.dram_tensor("band", (P, BSUM), FP16, kind="ExternalInput")
    m8_d = nc.dram_tensor("m8", (MJ, MI), FP8, kind="ExternalInput")
    g2_d = nc.dram_tensor("g2", (P, NJ * GW), FP16, kind="ExternalInput")
    gq_d = nc.dram_tensor("gq", (P, NJ * GW), FP16, kind="ExternalInput")
    gp_d = nc.dram_tensor("gp", (P, NJ * GW), FP16, kind="ExternalInput")
    es1b_d = nc.dram_tensor("es1b", (1, MI), FP16, kind="ExternalInput")
    es1bt_d = nc.dram_tensor("es1bt", (P, NIT), F32, kind="ExternalInput")
    es2m_d = nc.dram_tensor("es2m", (P, NJ), F32, kind="ExternalInput")
    out_d = nc.dram_tensor("out", (MI, GW), BF16, kind="ExternalOutput")

    # per-block count of P-chunks (suffix in c); 0 -> no scale needed
    pP = [sum(1 for c in range(NJ) if kp[c] <= it) for it in range(NIT)]

    with tile.TileContext(nc) as tc:
        with (
            tc.tile_pool(name="consts", bufs=1) as consts,
            tc.tile_pool(name="adj8p", bufs=3) as adj8p,
            tc.tile_pool(name="up", bufs=3) as up,
            tc.tile_pool(name="ntp", bufs=5) as ntp,
            tc.tile_pool(name="gsp", bufs=2) as gsp,
            tc.tile_pool(name="gqp", bufs=2) as gqp,
            tc.tile_pool(name="gpp", bufs=2) as gpp,
            tc.tile_pool(name="outp", bufs=1) as outp,
            tc.tile_pool(name="ps_acc", bufs=1, space="PSUM") as ps_acc,
        ):
            es2m = consts.tile([P, NJ], F32, tag="es2m")
            es1b = consts.tile([P, MI], FP16, tag="es1b")
            es1bt = consts.tile([P, NIT], F32, tag="es1bt")
            band = consts.tile([P, BSUM], FP16, tag="band")
            # band split: the first-processed chunks' slice lands first so
            # the fill isn't gated by the 1.6MB bulk transfer
            bt = offs[NJ - 2]
            if bt < BSUM:
                nc.scalar.dma_start(
                    out=band[:, bt:BSUM], in_=band_d[:, bt:BSUM]
                )
            nc.sync.dma_start(out=es2m[:], in_=es2m_d[:, :])
            nc.sync.dma_start(out=es1bt[:], in_=es1bt_d[:, :])
            if bt > 0:
                nc.sync.dma_start(out=band[:, 0:bt], in_=band_d[:, 0:bt])

            accs = [
                ps_acc.tile([P, 512], F32, tag=f"acc{b}", name=f"acc{b}")
                for b in range(8)
            ]

            def acc_slice(it):
                return accs[it // 2][:, (it % 2) * 256 : (it % 2) * 256 + GW]

            m8_r = m8_d.rearrange("(c p) m -> p c m", p=P)

            pend = []
            slabs = [None, None, None]
            cur8 = [None]
            group_of = {}
            for grp in groups:
                for c in grp:
                    group_of[c] = grp
            scaled = [False] * NIT

            def phase_a(c):
                if c % 8 == 7:
                    g8 = c // 8
                    gs = gsp.tile([P, 8 * GW], FP16, tag="gs")
                    gq = gqp.tile([P, 8 * GW], FP16, tag="gq")
                    gp = gpp.tile([P, 8 * GW], FP16, tag="gp")
                    sl = slice(g8 * 8 * GW, (g8 + 1) * 8 * GW)
                    nc.sync.dma_start(out=gs[:], in_=g2_d[:, sl])
                    nc.scalar.dma_start(out=gq[:], in_=gq_d[:, sl])
                    nc.sync.dma_start(out=gp[:], in_=gp_d[:, sl])
                    if c == NJ - 1:
                        nc.sync.dma_start(
                            out=es1b[:], in_=_broadcast_ap(es1b_d[:, :], P)
                        )
                    slabs[0], slabs[1], slabs[2] = gs, gq, gp
                grp = group_of[c]
                if c == grp[0]:
                    g8t = adj8p.tile([P, len(grp), MI], FP8, tag="adj8")
                    lo = min(grp)
                    eng8 = nc.sync if (grp[0] // 4) % 2 == 0 else nc.scalar
                    eng8.dma_start(
                        out=g8t[:], in_=m8_r[:, lo : lo + len(grp), :]
                    )
                    cur8[0] = (g8t, lo)
                pend.append((c, cur8[0], slabs[0], slabs[1], slabs[2]))

            def phase_c():
                c, (g8t, lo), gs, gq, gp = pend.pop(0)
                bw = kp[c] - kq[c]
                n_t = None
                if bw > 0:
                    w = bw * P
                    u_t = up.tile([P, BW * P], FP16, tag="u_t")
                    nc.vector.tensor_scalar(
                        out=u_t[:, :w],
                        in0=es1b[:, kq[c] * P : kp[c] * P],
                        scalar1=es2m[:, c : c + 1],
                        scalar2=1.0,
                        op0=OP.mult,
                        op1=OP.max,
                    )
                    n_t = ntp.tile([P, BW * P], FP16, tag="n_t")
                    nc.vector.tensor_tensor(
                        out=n_t[:, :w], in0=u_t[:, :w],
                        in1=band[:, offs[c] : offs[c] + w],
                        op=OP.mult,
                    )
                gsl = gs[:, (c % 8) * GW : (c % 8) * GW + GW]
                gql = gq[:, (c % 8) * GW : (c % 8) * GW + GW]
                gpl = gp[:, (c % 8) * GW : (c % 8) * GW + GW]
                for it in range(NIT):
                    is_p = it >= kp[c]
                    if not is_p and pP[it] > 0 and not scaled[it]:
                        # all P-chunks for this block done: fold es1b_i in
                        nc.vector.tensor_scalar(
                            out=acc_slice(it),
                            in0=acc_slice(it),
                            scalar1=es1bt[:, it : it + 1],
                            scalar2=None,
                            op0=OP.mult,
                        )
                        scaled[it] = True
                    if is_p:
                        stat = g8t[:, c - lo, it * P : (it + 1) * P]
                        mov = gpl
                    elif it < kq[c]:
                        stat = g8t[:, c - lo, it * P : (it + 1) * P]
                        mov = gql
                    else:
                        stat = n_t[:, (it - kq[c]) * P : (it - kq[c] + 1) * P]
                        mov = gsl
                    nc.tensor.matmul(
                        acc_slice(it),
                        stat,
                        mov,
                        start=(c == NJ - 1 and it % 2 == 0),
                        stop=(c == 0),
                        skip_group_check=True,
                    )

            order = list(range(NJ - 1, -1, -1))
            for idx, c in enumerate(order):
                phase_a(c)
                if idx >= LAG:
                    phase_c()
            while pend:
                phase_c()

            out_r = out_d.rearrange("(c p) f -> p c f", p=P)
            res = outp.tile([P, NIT, GW], BF16, tag="res")
            for g in range(4):
                for kk in range(4):
                    it = 4 * g + kk
                    if pP[it] > 0 and not scaled[it]:
                        nc.vector.tensor_scalar(
                            out=acc_slice(it), in0=acc_slice(it),
                            scalar1=es1bt[:, it : it + 1], scalar2=None,
                            op0=OP.mult,
                        )
                        scaled[it] = True
                    if it % 2 == 0:
                        nc.vector.tensor_copy(res[:, it, :], acc_slice(it))
                    else:
                        nc.scalar.copy(res[:, it, :], acc_slice(it))
                eng = nc.sync if g % 2 == 0 else nc.scalar
                eng.dma_start(
                    out=out_r[:, 4 * g : 4 * g + 4, :],
                    in_=res[:, 4 * g : 4 * g + 4, :],
                )

    nc.compile()
    return nc


def host_prepare(x, adj, W, a):
    h64 = x.astype(np.float64) @ W.astype(np.float64)
    s1 = h64 @ a[:F, 0].astype(np.float64)
    s2 = h64 @ a[F:, 0].astype(np.float64)
    es2a = np.exp(ALPHA * s2)
    es2m = np.exp((1.0 - ALPHA) * s2)
    es2f = np.exp(s2)
    g2 = np.empty((N, GW), np.float64)
    g2[:, :F] = h64
    g2[:, F] = 1.0
    gq64 = g2 * es2a[:, None]
    gp64 = g2 * es2f[:, None]
    g2 = g2.astype(np.float16)
    gq = gq64.astype(np.float16)
    gp = gp64.astype(np.float16)
    es1b16 = np.exp((1.0 - ALPHA) * s1).astype(np.float16)

    isort = np.argsort(s1, kind="stable")
    ilists = [isort[sl::4] for sl in range(4)]
    jsort = np.argsort(s2, kind="stable")
    jlists = [jsort[h::2] for h in range(2)]

    maskT = adj.T > 0

    kq_all, kp_all = [], []
    for h in range(2):
        es2m_h = es2m[jlists[h]].astype(np.float32)
        cmax = es2m_h.reshape(NJ, P).max(axis=1)
        cmin = es2m_h.reshape(NJ, P).min(axis=1)
        for sl in range(4):
            e1 = es1b16[ilists[sl]].astype(np.float32).reshape(NIT, P)
            bmax = e1.max(axis=1)
            bmin = e1.min(axis=1)
            kq_all.append((bmax[None, :] * cmax[:, None] <= 1.0).sum(axis=1))
            # P-suffix count: blocks with bmin * es2m_chunk_min >= 1
            cnt = (bmin[None, :] * cmin[:, None] >= 1.0).sum(axis=1)
            kp_all.append(NIT - cnt)
    kq = np.minimum.reduce(kq_all).astype(int)
    kp = np.maximum.reduce(kp_all).astype(int)
    kp = np.maximum(kp, kq)  # band must be non-negative
    kq_l, kp_l = kq.tolist(), kp.tolist()

    in_maps = []
    for c in range(NCORES):
        sl = c % 4
        h = c // 4
        il, jl = ilists[sl], jlists[h]
        mT = maskT[np.ix_(jl, il)]
        es2a_j = es2a[jl]
        mt = np.where(mT, es2a_j[:, None], 0.0).astype(np.float16)
        m8 = mT.astype(ml_dtypes.float8_e4m3)
        offs_l = []
        tot = 0
        for cc in range(NJ):
            offs_l.append(tot)
            tot += (kp_l[cc] - kq_l[cc]) * P
        band = np.zeros((P, max(tot, P)), np.float16)
        for cc in range(NJ):
            w = (kp_l[cc] - kq_l[cc]) * P
            if w > 0:
                band[:, offs_l[cc] : offs_l[cc] + w] = mt[
                    cc * P : (cc + 1) * P, kq_l[cc] * P : kp_l[cc] * P
                ]
        g2h = np.ascontiguousarray(
            g2[jl].reshape(NJ, P, GW).transpose(1, 0, 2).reshape(P, NJ * GW)
        )
        gqh = np.ascontiguousarray(
            gq[jl].reshape(NJ, P, GW).transpose(1, 0, 2).reshape(P, NJ * GW)
        )
        gph = np.ascontiguousarray(
            gp[jl].reshape(NJ, P, GW).transpose(1, 0, 2).reshape(P, NJ * GW)
        )
        es2mh = np.ascontiguousarray(es2m[jl].reshape(NJ, P).T.astype(np.float32))
        es1bth = np.ascontiguousarray(
            es1b16[il].astype(np.float32).reshape(NIT, P).T
        )
        in_maps.append(
            {
                "band": np.ascontiguousarray(band),
                "m8": np.ascontiguousarray(m8),
                "g2": g2h,
                "gq": gqh,
                "gp": gph,
                "es1b": es1b16[il].reshape(1, MI),
                "es1bt": es1bth,
                "es2m": es2mh,
            }
        )
    return in_maps, kq_l, kp_l, ilists


_NC_CACHE = {}


def kernel(x, adj, W, a, _trace=False):
    x = np.asarray(x)
    adj = np.asarray(adj)
    W = np.asarray(W)
    a = np.asarray(a)

    in_maps, kq, kp, ilists = host_prepare(x, adj, W, a)
    key = (tuple(kq), tuple(kp))
    if key not in _NC_CACHE:
        _NC_CACHE.clear()
        _NC_CACHE[key] = build_program(kq, kp)
    nc = _NC_CACHE[key]
    res = bass_utils.run_bass_kernel_spmd(
        nc, in_maps, core_ids=list(range(NCORES)), trace=_trace
    )
    nd = np.empty((N, GW), np.float32)
    for sl in range(4):
        nd[ilists[sl]] = (
            np.asarray(res.results[sl]["out"]).astype(np.float32)
            + np.asarray(res.results[sl + 4]["out"]).astype(np.float32)
        )
    hp = nd[:, :F] / nd[:, F : F + 1]
    out = np.where(hp > 0, hp, np.expm1(np.minimum(hp, 0.0))).astype(np.float32)
    if _trace:
        return out, res
    return out


# revision 13
# speedup vs baseline: 1.2060x; 1.2060x over previous
"""v28 Trainium2 Bass kernel for an attention-style graph convolution (GAT).

Full staircase split. With i sorted by s1 (stratified mod-4 over slabs) and
j sorted by s2 (stratified mod-2 over halves), each (chunk c, i-block it)
tile is one of three exact types:
  Q (u == 1,        bmax_es1b * es2m_c <= 1):  acc[it] += m8.T @ gQ_c
  P (u == es1b*es2m, bmin_es1b * es2m_c >= 1): acc[it] += m8.T @ gP_c,
     with the per-row factor es1b_i applied ONCE by an in-place DVE scale
     of the PSUM accumulator at the block's P->non-P transition
  band (otherwise): u = max(es1b*es2m_c, 1) (ts); n = u*mt (tt);
                    acc[it] += n.T @ g2_c
where mt = m*es2a_j (fp16, streamed only for the band), m8 = binary fp8,
gQ = es2a*g2, gP = es2f*g2. Chunks are processed in DESCENDING c (s2) so
each block's P-chunks come first (suffix in c = prefix in processing),
making the single in-place scale exact. Tables kq/kp are conservative
(min/max over the 8 cores) so one SPMD program serves all cores.
Host sums j-half core pairs, un-permutes rows, divides, applies elu.
"""

import ml_dtypes
import numpy as np

import concourse.bacc as bacc
import concourse.bass as bass
import concourse.mybir as mybir
import concourse.tile as tile
from concourse import bass_utils

F32 = mybir.dt.float32
BF16 = mybir.dt.bfloat16
FP16 = mybir.dt.float16
FP8 = mybir.dt.float8e4
OP = mybir.AluOpType

N = 8192
K = 256
F = 128
ALPHA = 0.2
NCORES = 8
MI = 2048
MJ = 4096
P = 128
NJ = MJ // P      # 32 j-chunks
NIT = MI // P     # 16 i-blocks
LAG = 4
GW = F + 1


def _broadcast_ap(row_ap, nparts):
    return bass.AP(
        tensor=row_ap.tensor,
        offset=row_ap.offset,
        ap=[[0, nparts]] + [list(d) for d in row_ap.ap],
    )


def band_offsets(kq, kp):
    offs, tot = [], 0
    for c in range(NJ):
        offs.append(tot)
        tot += (kp[c] - kq[c]) * P
    return offs, max(tot, P)


def m8_groups():
    # descending processing order; tapered group sizes at both ends
    gs, c = [], NJ - 1
    sizes = [1] + [4] * 7 + [1, 1, 1]
    for s in sizes:
        gs.append(list(range(c, c - s, -1)))
        c -= s
    assert c == -1
    return gs


def build_program(kq, kp):
    BW = max(1, max(p - q for p, q in zip(kp, kq)))
    offs, BSUM = band_offsets(kq, kp)
    groups = m8_groups()
    nc = bacc.Bacc("TRN2", target_bir_lowering=False)

    band_d = nc.dram_tensor("band", (P, BSUM), FP16, kind="ExternalInput")
    m8_d = nc.dram_tensor("m8", (MJ, MI), FP8, kind="ExternalInput")
    g2_d = nc.dram_tensor("g2", (P, NJ * GW), FP16, kind="ExternalInput")
    gq_d = nc.dram_tensor("gq", (P, NJ * GW), FP16, kind="ExternalInput")
    gp_d = nc.dram_tensor("gp", (P, NJ * GW), FP16, kind="ExternalInput")
    es1b_d = nc.dram_tensor("es1b", (1, MI), FP16, kind="ExternalInput")
    es1bt_d = nc.dram_tensor("es1bt", (P, NIT), F32, kind="ExternalInput")
    es2m_d = nc.dram_tensor("es2m", (P, NJ), F32, kind="ExternalInput")
    out_d = nc.dram_tensor("out", (MI, GW), BF16, kind="ExternalOutput")

    # per-block count of P-chunks (suffix in c); 0 -> no scale needed
    pP = [sum(1 for c in range(NJ) if kp[c] <= it) for it in range(NIT)]

    with tile.TileContext(nc) as tc:
        with (
            tc.tile_pool(name="consts", bufs=1) as consts,
            tc.tile_pool(name="adj8p", bufs=5) as adj8p,
            tc.tile_pool(name="up", bufs=3) as up,
            tc.tile_pool(name="ntp", bufs=5) as ntp,
            tc.tile_pool(name="gsp", bufs=2) as gsp,
            tc.tile_pool(name="gqp", bufs=2) as gqp,
            tc.tile_pool(name="gpp", bufs=2) as gpp,
            tc.tile_pool(name="outp", bufs=1) as outp,
            tc.tile_pool(name="ps_acc", bufs=1, space="PSUM") as ps_acc,
        ):
            es2m = consts.tile([P, NJ], F32, tag="es2m")
            es1b = consts.tile([P, MI], FP16, tag="es1b")
            es1bt = consts.tile([P, NIT], F32, tag="es1bt")
            band = consts.tile([P, BSUM], FP16, tag="band")
            # band split: the first-processed chunks' slice lands first so
            # the fill isn't gated by the 1.6MB bulk transfer
            bt = offs[NJ - 2]
            if bt < BSUM:
                nc.scalar.dma_start(
                    out=band[:, bt:BSUM], in_=band_d[:, bt:BSUM]
                )
            nc.sync.dma_start(out=es2m[:], in_=es2m_d[:, :])
            nc.sync.dma_start(out=es1bt[:], in_=es1bt_d[:, :])
            if bt > 0:
                nc.sync.dma_start(out=band[:, 0:bt], in_=band_d[:, 0:bt])

            accs = [
                ps_acc.tile([P, 512], F32, tag=f"acc{b}", name=f"acc{b}")
                for b in range(8)
            ]

            def acc_slice(it):
                return accs[it // 2][:, (it % 2) * 256 : (it % 2) * 256 + GW]

            m8_r = m8_d.rearrange("(c p) m -> p c m", p=P)

            pend = []
            slabs = [None, None, None]
            cur8 = [None]
            group_of = {}
            for grp in groups:
                for c in grp:
                    group_of[c] = grp
            scaled = [False] * NIT

            def phase_a(c):
                if c % 8 == 7:
                    g8 = c // 8
                    gs = gsp.tile([P, 8 * GW], FP16, tag="gs")
                    gq = gqp.tile([P, 8 * GW], FP16, tag="gq")
                    gp = gpp.tile([P, 8 * GW], FP16, tag="gp")
                    sl = slice(g8 * 8 * GW, (g8 + 1) * 8 * GW)
                    nc.sync.dma_start(out=gs[:], in_=g2_d[:, sl])
                    nc.scalar.dma_start(out=gq[:], in_=gq_d[:, sl])
                    nc.sync.dma_start(out=gp[:], in_=gp_d[:, sl])
                    if c == NJ - 1:
                        nc.sync.dma_start(
                            out=es1b[:], in_=_broadcast_ap(es1b_d[:, :], P)
                        )
                    slabs[0], slabs[1], slabs[2] = gs, gq, gp
                grp = group_of[c]
                if c == grp[0]:
                    g8t = adj8p.tile([P, len(grp), MI], FP8, tag="adj8")
                    lo = min(grp)
                    eng8 = nc.sync if (grp[0] // 4) % 2 == 0 else nc.scalar
                    eng8.dma_start(
                        out=g8t[:], in_=m8_r[:, lo : lo + len(grp), :]
                    )
                    cur8[0] = (g8t, lo)
                pend.append((c, cur8[0], slabs[0], slabs[1], slabs[2]))

            def phase_c():
                c, (g8t, lo), gs, gq, gp = pend.pop(0)
                bw = kp[c] - kq[c]
                n_t = None
                if bw > 0:
                    w = bw * P
                    u_t = up.tile([P, BW * P], FP16, tag="u_t")
                    nc.vector.tensor_scalar(
                        out=u_t[:, :w],
                        in0=es1b[:, kq[c] * P : kp[c] * P],
                        scalar1=es2m[:, c : c + 1],
                        scalar2=1.0,
                        op0=OP.mult,
                        op1=OP.max,
                    )
                    n_t = ntp.tile([P, BW * P], FP16, tag="n_t")
                    nc.vector.tensor_tensor(
                        out=n_t[:, :w], in0=u_t[:, :w],
                        in1=band[:, offs[c] : offs[c] + w],
                        op=OP.mult,
                    )
                gsl = gs[:, (c % 8) * GW : (c % 8) * GW + GW]
                gql = gq[:, (c % 8) * GW : (c % 8) * GW + GW]
                gpl = gp[:, (c % 8) * GW : (c % 8) * GW + GW]
                for it in range(NIT):
                    is_p = it >= kp[c]
                    if not is_p and pP[it] > 0 and not scaled[it]:
                        # all P-chunks for this block done: fold es1b_i in
                        nc.vector.tensor_scalar(
                            out=acc_slice(it),
                            in0=acc_slice(it),
                            scalar1=es1bt[:, it : it + 1],
                            scalar2=None,
                            op0=OP.mult,
                        )
                        scaled[it] = True
                    if is_p:
                        stat = g8t[:, c - lo, it * P : (it + 1) * P]
                        mov = gpl
                    elif it < kq[c]:
                        stat = g8t[:, c - lo, it * P : (it + 1) * P]
                        mov = gql
                    else:
                        stat = n_t[:, (it - kq[c]) * P : (it - kq[c] + 1) * P]
                        mov = gsl
                    nc.tensor.matmul(
                        acc_slice(it),
                        stat,
                        mov,
                        start=(c == NJ - 1 and it % 2 == 0),
                        stop=(c == 0),
                        skip_group_check=True,
                    )

            order = list(range(NJ - 1, -1, -1))
            for idx, c in enumerate(order):
                phase_a(c)
                if idx >= LAG:
                    phase_c()
            while pend:
                phase_c()

            out_r = out_d.rearrange("(c p) f -> p c f", p=P)
            res = outp.tile([P, NIT, GW], BF16, tag="res")
            for g in range(4):
                for kk in range(4):
                    it = 4 * g + kk
                    if pP[it] > 0 and not scaled[it]:
                        nc.vector.tensor_scalar(
                            out=acc_slice(it), in0=acc_slice(it),
                            scalar1=es1bt[:, it : it + 1], scalar2=None,
                            op0=OP.mult,
                        )
                        scaled[it] = True
                    if it % 2 == 0:
                        nc.vector.tensor_copy(res[:, it, :], acc_slice(it))
                    else:
                        nc.scalar.copy(res[:, it, :], acc_slice(it))
                eng = nc.sync if g % 2 == 0 else nc.scalar
                eng.dma_start(
                    out=out_r[:, 4 * g : 4 * g + 4, :],
                    in_=res[:, 4 * g : 4 * g + 4, :],
                )

    nc.compile()
    return nc


def host_prepare(x, adj, W, a):
    h64 = x.astype(np.float64) @ W.astype(np.float64)
    s1 = h64 @ a[:F, 0].astype(np.float64)
    s2 = h64 @ a[F:, 0].astype(np.float64)
    es2a = np.exp(ALPHA * s2)
    es2m = np.exp((1.0 - ALPHA) * s2)
    es2f = np.exp(s2)
    g2 = np.empty((N, GW), np.float64)
    g2[:, :F] = h64
    g2[:, F] = 1.0
    gq64 = g2 * es2a[:, None]
    gp64 = g2 * es2f[:, None]
    g2 = g2.astype(np.float16)
    gq = gq64.astype(np.float16)
    gp = gp64.astype(np.float16)
    es1b16 = np.exp((1.0 - ALPHA) * s1).astype(np.float16)

    isort = np.argsort(s1, kind="stable")
    ilists = [isort[sl::4] for sl in range(4)]
    jsort = np.argsort(s2, kind="stable")
    jlists = [jsort[h::2] for h in range(2)]

    maskT = adj.T > 0

    kq_all, kp_all = [], []
    for h in range(2):
        es2m_h = es2m[jlists[h]].astype(np.float32)
        cmax = es2m_h.reshape(NJ, P).max(axis=1)
        cmin = es2m_h.reshape(NJ, P).min(axis=1)
        for sl in range(4):
            e1 = es1b16[ilists[sl]].astype(np.float32).reshape(NIT, P)
            bmax = e1.max(axis=1)
            bmin = e1.min(axis=1)
            kq_all.append((bmax[None, :] * cmax[:, None] <= 1.0).sum(axis=1))
            # P-suffix count: blocks with bmin * es2m_chunk_min >= 1
            cnt = (bmin[None, :] * cmin[:, None] >= 1.0).sum(axis=1)
            kp_all.append(NIT - cnt)
    kq = np.minimum.reduce(kq_all).astype(int)
    kp = np.maximum.reduce(kp_all).astype(int)
    kp = np.maximum(kp, kq)  # band must be non-negative
    kq_l, kp_l = kq.tolist(), kp.tolist()

    in_maps = []
    for c in range(NCORES):
        sl = c % 4
        h = c // 4
        il, jl = ilists[sl], jlists[h]
        mT = maskT[np.ix_(jl, il)]
        es2a_j = es2a[jl]
        mt = np.where(mT, es2a_j[:, None], 0.0).astype(np.float16)
        m8 = mT.astype(ml_dtypes.float8_e4m3)
        offs_l = []
        tot = 0
        for cc in range(NJ):
            offs_l.append(tot)
            tot += (kp_l[cc] - kq_l[cc]) * P
        band = np.zeros((P, max(tot, P)), np.float16)
        for cc in range(NJ):
            w = (kp_l[cc] - kq_l[cc]) * P
            if w > 0:
                band[:, offs_l[cc] : offs_l[cc] + w] = mt[
                    cc * P : (cc + 1) * P, kq_l[cc] * P : kp_l[cc] * P
                ]
        g2h = np.ascontiguousarray(
            g2[jl].reshape(NJ, P, GW).transpose(1, 0, 2).reshape(P, NJ * GW)
        )
        gqh = np.ascontiguousarray(
            gq[jl].reshape(NJ, P, GW).transpose(1, 0, 2).reshape(P, NJ * GW)
        )
        gph = np.ascontiguousarray(
            gp[jl].reshape(NJ, P, GW).transpose(1, 0, 2).reshape(P, NJ * GW)
        )
        es2mh = np.ascontiguousarray(es2m[jl].reshape(NJ, P).T.astype(np.float32))
        es1bth = np.ascontiguousarray(
            es1b16[il].astype(np.float32).reshape(NIT, P).T
        )
        in_maps.append(
            {
                "band": np.ascontiguousarray(band),
                "m8": np.ascontiguousarray(m8),
                "g2": g2h,
                "gq": gqh,
                "gp": gph,
                "es1b": es1b16[il].reshape(1, MI),
                "es1bt": es1bth,
                "es2m": es2mh,
            }
        )
    return in_maps, kq_l, kp_l, ilists


_NC_CACHE = {}


def kernel(x, adj, W, a, _trace=False):
    x = np.asarray(x)
    adj = np.asarray(adj)
    W = np.asarray(W)
    a = np.asarray(a)

    in_maps, kq, kp, ilists = host_prepare(x, adj, W, a)
    key = (tuple(kq), tuple(kp))
    if key not in _NC_CACHE:
        _NC_CACHE.clear()
        _NC_CACHE[key] = build_program(kq, kp)
    nc = _NC_CACHE[key]
    res = bass_utils.run_bass_kernel_spmd(
        nc, in_maps, core_ids=list(range(NCORES)), trace=_trace
    )
    nd = np.empty((N, GW), np.float32)
    for sl in range(4):
        nd[ilists[sl]] = (
            np.asarray(res.results[sl]["out"]).astype(np.float32)
            + np.asarray(res.results[sl + 4]["out"]).astype(np.float32)
        )
    hp = nd[:, :F] / nd[:, F : F + 1]
    out = np.where(hp > 0, hp, np.expm1(np.minimum(hp, 0.0))).astype(np.float32)
    if _trace:
        return out, res
    return out
